# revision 24
# baseline (speedup 1.0000x reference)
"""KNN anomaly-score kernel for Trainium2 (8 NeuronCores, Bass/Tile).

Problem: features [B=1024, D=768], memory_bank [N=50000, D=768], k=9.
anomaly_score[b] = mean of the k smallest Euclidean distances from
features[b] to the memory bank rows.

Strategy (per the sharding hint): shard memory-bank rows across the 8
cores.  Each core computes its [B, N/8] block of a selection score
v = f.m - |m|^2/2 + C on the TensorEngine as ONE fp8-e4m3 DoubleRow
GEMM (two K=128 subtiles per instruction, 2x column rate), with the
m-norm folded into the GEMM itself: data dimension D-1 is dropped from
the cross term and its rows repurposed as an augment pair
(features row D-1 := 8.0, bank row D-1 := fp8((C - |m|^2/2)/8),
C = 384).  The per-row |f|^2/2 term is constant along the selection
axis, so it never needs to reach the device - the host adds the exact
x_sq back when converting candidate v values to distances:
d^2 = x_sq + 2C - 2v.

Error budget on v (= -d^2/2 + const, d ~ 39): fp8 rounding of the
cross term ~0.7, the dropped dim-767 cross term ~1.0, fp8 encoding of
the centered m-norm ~0.6 => ~1.4 total, i.e. ~2e-3 relative on d -
well inside the 2e-2 gate.

Selection: for each column chunk (512/1024/tail) the DVE MAX8
instruction extracts the chunk's top-8 v values straight out of PSUM.
The device returns all chunk candidates [B, 8*nchunks]; the host
gathers the 8 cores' candidates and reduces to the global top-k.  A
true top-k member can be missing only if >=8 elements of its chunk
rank above it, which forces >=8 of the observed top-k to come from
that single chunk - the host detects exactly that condition and
recomputes the affected rows with numpy.

Orchestration (from trace analysis; the GEMM floor at the measured
157 TF/s fp8 DoubleRow peak is 62.5us/core and ~12us of the graded
window is fixed framework preamble/epilogue, so the wins are in the
startup ramp and tail):
- every first-wave transfer is contiguous-per-partition on a HWDGE
  queue: chunk 0 gets its own exact-width SBUF tile (3KB lines run
  ~2.5x faster than the 512B lines a partial-width tile slice
  produces), and the feature tile arrives as 3 whole k-pair slices
  (2KB lines) - the old layout put the k45 pair on the gpsimd SWDGE
  path, which landed ~4us late and stalled the PE 3.2us;
- chunk 0 runs j outer / m inner over two m-groups of 4 (the 4 PSUM
  bufs), so the PE starts on k-pair 0 as soon as it lands instead of
  waiting for the full feature tile;
- warm-up matmuls on a zero tile keep the PE's p-state ramping until
  the first-wave DMAs land;
- the output tile is chunk-major so each chunk's candidate slice DMAs
  out mid-run; only the last chunk's slice gates the end.
"""

import functools
import sys

sys.path.insert(0, "/opt/trn_rl_repo")

import numpy as np

P = 128
NCORES = 8
C_M = 384.0  # centering constant for the fp8 m-norm row: v = f.m + C_M - |m|^2/2
N_WARMUP = 7


def _ceil_to(x, m):
    return (x + m - 1) // m * m


def _chunk_plan(NPAD):
    """Chunk widths: a 512 starter (small gating first-wave DMA), a 768
    second chunk (its DMA queues behind the first wave on the sync ring -
    768 lands ~1.3us earlier than a full 1024 and removes a jittery PE
    stall), full 1024s, then a [~768, 106] tail so the last chunk's MAX8
    and candidate DMA on the critical tail are small (all >=8 for MAX8)."""
    if NPAD <= 1024:
        return [NPAD]
    out = [512]
    rem = NPAD - 512
    if rem >= 768 + 8:
        out.append(768)
        rem -= 768
    nf = rem // 1024
    rem -= nf * 1024
    out += [1024] * nf
    if rem == 0:
        pass
    elif rem > 520:
        out += [rem - 106, 106]
    elif rem >= 8:
        out.append(rem)
    else:
        out[-1] -= 8 - rem
        out.append(8)
    return out


@functools.lru_cache(maxsize=4)
def _build(B, D, NPAD):
    """Build (and finalize) the SPMD Bass module for one core's shard."""
    from contextlib import ExitStack

    import concourse.tile as tile
    from concourse import bacc, mybir

    f32 = mybir.dt.float32
    bf16 = mybir.dt.bfloat16
    fp8 = mybir.dt.float8e4

    KT = D // P
    MT = B // P
    assert D % P == 0 and B % P == 0 and NPAD >= 1024
    assert KT % 2 == 0, "DoubleRow consumes K=128 subtiles in pairs"
    KP = KT // 2
    widths = _chunk_plan(NPAD)
    NCH = len(widths)
    CW = 8 * NCH  # candidates per row per core

    nc = bacc.Bacc(
        "TRN2", target_bir_lowering=False, debug=False, num_devices=NCORES
    )

    # chunk 1 can be split into two half-width DRAM params so its two DMAs
    # ride both HWDGE rings in parallel; with f[k45] moved off the sync ring
    # chunk 1 lands in time as one transfer, so the split stays off
    C1SPLIT = False
    f_t = nc.declare_dram_parameter("f_t", [P, KT * B], fp8, isOutput=False)
    bsegs = []
    for i, w in enumerate(widths):
        if i == 1 and C1SPLIT:
            bsegs.append(
                (
                    nc.declare_dram_parameter("bseg1a", [P, KT * 512], fp8, isOutput=False),
                    nc.declare_dram_parameter("bseg1b", [P, KT * 512], fp8, isOutput=False),
                )
            )
        else:
            bsegs.append(
                nc.declare_dram_parameter(f"bseg{i}", [P, KT * w], fp8, isOutput=False)
            )
    out = nc.declare_dram_parameter("cand", [P, MT * CW], f32, isOutput=True)

    # f is stored half-major in DRAM ([P, NH, KT, BH] flattened) so every
    # (B-half, k-pair) slice is one contiguous-per-partition DMA; the m-group
    # loop consumes exactly one half per group, so the halves stream in the
    # order the PE needs them with no oversized gating transfer
    BH = 4 * P if B % (4 * P) == 0 else B
    NH = B // BH
    MH = BH // P

    with tile.TileContext(nc) as tc, ExitStack() as ctx:
        cpool = ctx.enter_context(tc.tile_pool(name="const", bufs=1))
        ppool = ctx.enter_context(tc.tile_pool(name="psum", bufs=4, space="PSUM"))

        f_view = f_t.rearrange("p (h kt b) -> p h kt b", h=NH, kt=KT)
        bviews = [
            tuple(h.rearrange("p (kt n) -> p kt n", kt=KT) for h in s)
            if isinstance(s, tuple)
            else s.rearrange("p (kt n) -> p kt n", kt=KT)
            for s in bsegs
        ]

        # PE warm-up during the initial DMA wait: garbage matmuls on a
        # zeroed tile keep the clock ramping until real work arrives.
        warm = cpool.tile([P, 512], bf16, tag="warm")
        nc.gpsimd.memset(warm[:], 0.0)
        wpsum = ppool.tile([P, 1024], f32, tag="pt")  # borrow a pt slot
        for _ in range(N_WARMUP):
            nc.tensor.matmul(
                wpsum[:, :512], lhsT=warm[:, :P], rhs=warm[:], start=True, stop=True
            )

        # First wave, all contiguous-per-partition on the two HWDGE
        # queues:  sync: chunk0, f[k45]  /  scalar: f[k01], f[k23].
        # Later chunks alternate sync/scalar and queue FIFO behind.
        # per chunk: list of (tile, col_lo) pieces, each an exact-width SBUF
        # tile so every DMA destination is contiguous per partition
        btiles = []
        for i, w in enumerate(widths):
            if i == 1 and C1SPLIT:
                btiles.append(
                    [
                        (cpool.tile([P, KT, 512], fp8, tag="bt1a", name="bt1a"), 0),
                        (cpool.tile([P, KT, 512], fp8, tag="bt1b", name="bt1b"), 512),
                    ]
                )
            else:
                btiles.append(
                    [(cpool.tile([P, KT, w], fp8, tag=f"bt{i}", name=f"bt{i}"), 0)]
                )
        ftile = cpool.tile([P, NH, KT, BH], fp8, tag="ft", name="ft")

        # sync ring:   chunk0, f[k45,h0], f[k45,h1], chunk1, chunk3, ...
        # scalar ring: f[k01,h0], f[k23,h0], f[k01,h1], f[k23,h1], chunk2, ...
        # Each f transfer is one (B-half, k-pair) slice (~131KB, 1KB lines),
        # ordered exactly as the grouped chunk-0 loop consumes them: group 0
        # (m0-3) needs only the h0 slices, which all land by ~11.5us, so the
        # PE starts on chunk0+k01 with no further feature stalls.  Measured
        # orderings with k45 behind the other f slices on one ring stalled
        # the PE 1.6-2.7us.
        nc.sync.dma_start(btiles[0][0][0][:], bviews[0][:])
        for h in range(NH):
            for j in range(KP - 1):
                nc.scalar.dma_start(
                    ftile[:, h, 2 * j : 2 * j + 2, :],
                    f_view[:, h, 2 * j : 2 * j + 2, :],
                )
        for h in range(NH):
            jl = 2 * (KP - 1)
            nc.sync.dma_start(
                ftile[:, h, jl : jl + 2, :], f_view[:, h, jl : jl + 2, :]
            )
        for i in range(1, NCH):
            if i == 1 and C1SPLIT:
                nc.sync.dma_start(btiles[1][0][0][:], bviews[1][0][:])
                nc.scalar.dma_start(btiles[1][1][0][:], bviews[1][1][:])
            else:
                q = nc.sync if i % 2 == 1 else nc.scalar
                q.dma_start(btiles[i][0][0][:], bviews[i][:])

        # chunk-major output tile: chunk ci's 8*MT candidate columns are
        # complete right after its m-loop, so each chunk's slice DMAs out
        # mid-run and only the last chunk's slice gates the end
        otile = cpool.tile([P, NCH * MT * 8], f32, tag="cand", name="cand")

        def chalves(W):
            out_, lo = [], 0
            while lo < W:
                out_.append((lo, min(512, W - lo)))
                lo += 512
            return out_

        def mm(pt, ci, j, m, hlo, hw, start, stop):
            tile_, lo = next(
                (t, lo)
                for t, lo in reversed(btiles[ci])
                if lo <= hlo and hlo + hw <= lo + t.shape[2]
            )
            ml = m % MH
            nc.tensor.matmul(
                pt[:, hlo : hlo + hw],
                lhsT=ftile[:, m // MH, 2 * j : 2 * j + 2, ml * P : (ml + 1) * P],
                rhs=tile_[:, 2 * j : 2 * j + 2, hlo - lo : hlo - lo + hw],
                start=start,
                stop=stop,
                perf_mode=mybir.MatmulPerfMode.DoubleRow,
            )

        # chunk 0: j outer / m inner in m-groups of 4 (the PSUM bufs),
        # matching the k-pair arrival order so the PE starts as soon as
        # chunk0 + f[k01] land instead of waiting for the whole ftile.
        W0 = widths[0]
        GRP = min(MH, 4)  # psum pool has 4 bufs
        pts0 = {}
        for g in range(0, MT, GRP):
            ms = range(g, min(g + GRP, MT))
            for j in range(KP):
                for m in ms:
                    if j == 0:
                        pts0[m] = ppool.tile([P, 1024], f32, tag="pt", name=f"pt0_{m}")
                    for hlo, hw in chalves(W0):
                        mm(pts0[m], 0, j, m, hlo, hw, j == 0, j == KP - 1)
            for m in ms:
                nc.vector.max(otile[:, m * 8 : m * 8 + 8], pts0[m][:, :W0])
        nc.sync.dma_start(out[:, : MT * 8], otile[:, : MT * 8])

        # chunks 1+: plain per-m j-loop (data long since resident); each
        # chunk's candidate slice DMAs out mid-run so only the last (small)
        # slice gates the end
        for ci in range(1, NCH):
            W = widths[ci]
            for m in range(MT):
                pt = ppool.tile([P, 1024], f32, tag="pt", name=f"pt{ci}_{m}")
                for j in range(KP):
                    for hlo, hw in chalves(W):
                        mm(pt, ci, j, m, hlo, hw, j == 0, j == KP - 1)
                base = ci * MT * 8 + m * 8
                nc.vector.max(otile[:, base : base + 8], pt[:, :W])
            oq = nc.sync if ci % 2 == 0 else nc.scalar
            oq.dma_start(
                out[:, ci * MT * 8 : (ci + 1) * MT * 8],
                otile[:, ci * MT * 8 : (ci + 1) * MT * 8],
            )

    nc.finalize()
    return nc


def _seg(bT, KT, off, W):
    """Columns [off, off+W) of [D, cols] -> tile layout [P, KT*W]."""
    D = bT.shape[0]
    P_ = P
    s = bT[:, off : off + W].reshape(KT, P_, W)
    return np.ascontiguousarray(s.transpose(1, 0, 2).reshape(P_, KT * W))


def _tile_f(fT, KT):
    """[D, B] -> half-major tile layout [P, NH*KT*BH] (d = kt*P + p): B is
    split into 512-column halves so each (half, k-pair) DRAM slice is one
    contiguous-per-partition DMA."""
    B = fT.shape[1]
    BH = 4 * P if B % (4 * P) == 0 else B
    return np.concatenate(
        [_seg(fT, KT, h, BH) for h in range(0, B, BH)], axis=1
    )


def _unseg(arr, KT, W):
    """Tile layout [P, KT*W] -> [D, W] (inverse of _seg, for the sim check)."""
    return arr.reshape(P, KT, W).transpose(1, 0, 2).reshape(KT * P, W)


def _tile_b(bT, KT, widths):
    """[D, NPAD] -> dict of per-chunk tile-layout arrays bseg{i}."""
    out = {}
    off = 0
    for i, w in enumerate(widths):
        out[f"bseg{i}"] = _seg(bT, KT, off, w)
        off += w
    return out


def _shard_geom(N):
    NSH = -(-N // NCORES)
    NPAD = max(NSH, 1024)
    return NSH, NPAD, _chunk_plan(NPAD)


def _host_prep(features, memory_bank):
    """Shard + lay out inputs for the 8 cores."""
    import ml_dtypes

    f8 = ml_dtypes.float8_e4m3
    B, D = features.shape
    N = memory_bank.shape[0]
    NSH, NPAD, widths = _shard_geom(N)
    KT = D // P

    # Cross term uses data dims 0..D-2 only; row D-1 carries the m-norm:
    #   v = f[:D-1].m[:D-1] + 8 * fp8((C_M - |m|^2/2) / 8)
    fT = np.ascontiguousarray(features.T).astype(f8)
    fT[D - 1] = f8(8.0)
    x_sq = np.einsum("bd,bd->b", features, features, dtype=np.float32)
    f_tiled = _tile_f(fT, KT)

    msq = np.einsum("nd,nd->n", memory_bank, memory_bank, dtype=np.float32)

    in_maps = []
    for i in range(NCORES):
        lo = i * NSH
        hi = min(lo + NSH, N)
        n_i = hi - lo
        bT = np.zeros((D, NPAD), f8)
        bT[:, :n_i] = memory_bank[lo:hi].T.astype(f8)
        q_m = np.full(NPAD, -240.0, np.float32)  # pads: v = -1920, never top-8
        q_m[:n_i] = (C_M - 0.5 * msq[lo:hi]) / 8.0
        bT[D - 1] = q_m.astype(f8)
        im = {"f_t": f_tiled}
        im.update(_tile_b(bT, KT, widths))
        in_maps.append(im)
    return in_maps, NPAD, x_sq, msq


# test.py can flip these to get a profiled run
TRACE = False
LAST_RESULT = None
N_RECOMPUTED = 0


def _install_ntff_hook():
    """This container's `antenv` lacks `axon_hooks`; synthesize it so
    run_bass_kernel_spmd(trace=True) can profile via the axon .so."""
    import sys as _sys

    if "antenv.axon_hooks" in _sys.modules:
        return
    import contextlib, ctypes, types

    mod = types.ModuleType("antenv.axon_hooks")
    mod._hook = None
    mod.set_axon_ntff_profile_hook = lambda h: setattr(mod, "_hook", h)
    mod.get_axon_ntff_profile_hook = lambda: mod._hook

    so_path = "/opt/axon/libaxon_pjrt.so"
    try:
        lib = ctypes.CDLL(so_path)
        lib.axon_start_nrt_profile.argtypes = [
            ctypes.POINTER(ctypes.c_int64),
            ctypes.c_size_t,
        ]
        lib.axon_start_nrt_profile.restype = ctypes.c_int64
        lib.axon_stop_nrt_profile.argtypes = [ctypes.c_char_p]
        lib.axon_stop_nrt_profile.restype = ctypes.c_int64

        @contextlib.contextmanager
        def _hook(output_dir, device_ids):
            import jax

            jax.devices()
            if device_ids:
                ids = (ctypes.c_int64 * len(device_ids))(*device_ids)
                rc = lib.axon_start_nrt_profile(ids, len(device_ids))
            else:
                rc = lib.axon_start_nrt_profile(None, 0)
            if rc != 0:
                raise RuntimeError(f"axon_start_nrt_profile rc={rc}")
            try:
                yield
            finally:
                n = lib.axon_stop_nrt_profile(str(output_dir).encode())
                print(f"profile: {n} file(s) written to {output_dir}")

        mod._hook = _hook
    except (OSError, AttributeError):
        pass

    import antenv

    _sys.modules["antenv.axon_hooks"] = mod
    antenv.axon_hooks = mod


def _exact_row_scores(features, memory_bank, rows, kk):
    """Exact numpy top-k mean distance for a few suspect rows."""
    f = features[rows]  # [R, D]
    d2 = (
        np.einsum("rd,rd->r", f, f)[:, None]
        + np.einsum("nd,nd->n", memory_bank, memory_bank)[None, :]
        - 2.0 * (f @ memory_bank.T)
    )
    d2k = np.sort(d2, axis=1)[:, :kk]
    return np.sqrt(np.maximum(d2k, 0.0)).mean(axis=1)


def kernel(features, memory_bank, k):
    global LAST_RESULT, N_RECOMPUTED
    from concourse.bass_utils import run_bass_kernel_spmd

    features = np.asarray(features, dtype=np.float32)
    memory_bank = np.asarray(memory_bank, dtype=np.float32)
    B, D = features.shape
    N = memory_bank.shape[0]
    kk = min(int(k), N)
    if kk <= 0:
        # mean over an empty candidate set (matches jnp.mean of empty)
        return np.full(B, np.nan, np.float32)

    in_maps, NPAD, x_sq, msq = _host_prep(features, memory_bank)
    nc = _build(B, D, NPAD)

    if TRACE:
        _install_ntff_hook()
    res = run_bass_kernel_spmd(nc, in_maps, list(range(NCORES)), trace=TRACE)
    LAST_RESULT = res

    # gather per-(core, chunk) top-8 candidates; larger v = closer
    MT = B // P
    v = np.concatenate(
        [_untile_cand(res.results[i]["cand"], MT) for i in range(NCORES)], axis=1
    )  # [B, NCORES * 8 * nchunks]
    return _finalize(v, x_sq, features, memory_bank, kk)


def _untile_cand(arr, MT):
    """Device cand layout [P, NCH*MT*8] (chunk-major) -> [B, NCH*8]."""
    NCH = arr.shape[1] // (MT * 8)
    return arr.reshape(P, NCH, MT, 8).transpose(2, 0, 1, 3).reshape(MT * P, NCH * 8)


def _finalize(v, x_sq, features, memory_bank, kk):
    """Reduce the per-(core, chunk) top-8 candidates to the final scores."""
    global N_RECOMPUTED
    kk_c = min(kk, v.shape[1])
    order = np.argsort(-v, axis=1)[:, :kk_c]  # observed top-k candidates
    vk = np.take_along_axis(v, order, axis=1)
    # v = f.m_trunc + C_M - |m|^2/2  =>  d^2 = x_sq + 2*C_M - 2*v
    d = np.sqrt(np.maximum(x_sq[:, None] + 2.0 * C_M - 2.0 * vk, 0.0))
    scores = d.mean(axis=1).astype(np.float32)

    # A true top-k member can only be missing if >=8 elements of its
    # column chunk outrank it; then >=8 of the observed top-k come
    # from that chunk (index group of 8).  Recompute such rows exactly.
    N_RECOMPUTED = 0
    if kk >= 9:
        if kk > v.shape[1]:  # more than the candidate pool: all rows exact
            suspects = np.arange(v.shape[0])
        else:
            grp = np.sort(order // 8, axis=1)
            same8 = (grp[:, 7:] == grp[:, : grp.shape[1] - 7]).any(axis=1)
            suspects = np.nonzero(same8)[0]
        if suspects.size:
            N_RECOMPUTED = suspects.size
            scores[suspects] = _exact_row_scores(
                features, memory_bank, suspects, kk
            ).astype(np.float32)

    return scores


# revision 25
# speedup vs baseline: 1.0048x; 1.0048x over previous
"""KNN anomaly-score kernel for Trainium2 (8 NeuronCores, Bass/Tile).

Problem: features [B=1024, D=768], memory_bank [N=50000, D=768], k=9.
anomaly_score[b] = mean of the k smallest Euclidean distances from
features[b] to the memory bank rows.

Strategy (per the sharding hint): shard memory-bank rows across the 8
cores.  Each core computes its [B, N/8] block of a selection score
v = f.m - |m|^2/2 + C on the TensorEngine as ONE fp8-e4m3 DoubleRow
GEMM (two K=128 subtiles per instruction, 2x column rate), with the
m-norm folded into the GEMM itself: data dimension D-1 is dropped from
the cross term and its rows repurposed as an augment pair
(features row D-1 := 8.0, bank row D-1 := fp8((C - |m|^2/2)/8),
C = 384).  The per-row |f|^2/2 term is constant along the selection
axis, so it never needs to reach the device - the host adds the exact
x_sq back when converting candidate v values to distances:
d^2 = x_sq + 2C - 2v.

Error budget on v (= -d^2/2 + const, d ~ 39): fp8 rounding of the
cross term ~0.7, the dropped dim-767 cross term ~1.0, fp8 encoding of
the centered m-norm ~0.6 => ~1.4 total, i.e. ~2e-3 relative on d -
well inside the 2e-2 gate.

Selection: for each column chunk (512/1024/tail) the DVE MAX8
instruction extracts the chunk's top-8 v values straight out of PSUM.
The device returns all chunk candidates [B, 8*nchunks]; the host
gathers the 8 cores' candidates and reduces to the global top-k.  A
true top-k member can be missing only if >=8 elements of its chunk
rank above it, which forces >=8 of the observed top-k to come from
that single chunk - the host detects exactly that condition and
recomputes the affected rows with numpy.

Orchestration (from trace analysis; the GEMM floor at the measured
157 TF/s fp8 DoubleRow peak is 62.5us/core and ~12us of the graded
window is fixed framework preamble/epilogue, so the wins are in the
startup ramp and tail):
- every first-wave transfer is contiguous-per-partition on a HWDGE
  queue: chunk 0 gets its own exact-width SBUF tile (3KB lines run
  ~2.5x faster than the 512B lines a partial-width tile slice
  produces), and the feature tile arrives as 3 whole k-pair slices
  (2KB lines) - the old layout put the k45 pair on the gpsimd SWDGE
  path, which landed ~4us late and stalled the PE 3.2us;
- chunk 0 runs j outer / m inner over two m-groups of 4 (the 4 PSUM
  bufs), so the PE starts on k-pair 0 as soon as it lands instead of
  waiting for the full feature tile;
- warm-up matmuls on a zero tile keep the PE's p-state ramping until
  the first-wave DMAs land;
- the output tile is chunk-major so each chunk's candidate slice DMAs
  out mid-run; only the last chunk's slice gates the end.
"""

import functools
import sys

sys.path.insert(0, "/opt/trn_rl_repo")

import numpy as np

P = 128
NCORES = 8
C_M = 384.0  # centering constant for the fp8 m-norm row: v = f.m + C_M - |m|^2/2
N_WARMUP = 7


def _ceil_to(x, m):
    return (x + m - 1) // m * m


def _chunk_plan(NPAD):
    """Chunk widths: a 512 starter (small gating first-wave DMA), a 768
    second chunk (its DMA queues behind the first wave on the sync ring -
    768 lands ~1.3us earlier than a full 1024 and removes a jittery PE
    stall), full 1024s, then a [~768, 106] tail so the last chunk's MAX8
    and candidate DMA on the critical tail are small (all >=8 for MAX8)."""
    if NPAD <= 1024:
        return [NPAD]
    out = [512]
    rem = NPAD - 512
    if rem >= 768 + 8:
        out.append(768)
        rem -= 768
    nf = rem // 1024
    rem -= nf * 1024
    out += [1024] * nf
    if rem == 0:
        pass
    elif rem > 520:
        # The sub-1024 chunk goes mid-run, not next to the tail: MAX8 on a
        # W-wide chunk costs ~1.04W+250ns/m vs the PE's ~1.25W+150ns/m, so
        # chunks under ~900 wide let the DVE drift behind the PE; a
        # following 1024 chunk absorbs that drift, but if the drift lands
        # right before the 106 tail the PE stalls ~0.4us on PSUM reuse.
        out.insert(2 + nf // 2, rem - 106)
        out.append(106)
    elif rem >= 8:
        out.append(rem)
    else:
        out[-1] -= 8 - rem
        out.append(8)
    return out


@functools.lru_cache(maxsize=4)
def _build(B, D, NPAD):
    """Build (and finalize) the SPMD Bass module for one core's shard."""
    from contextlib import ExitStack

    import concourse.tile as tile
    from concourse import bacc, mybir

    f32 = mybir.dt.float32
    bf16 = mybir.dt.bfloat16
    fp8 = mybir.dt.float8e4

    KT = D // P
    MT = B // P
    assert D % P == 0 and B % P == 0 and NPAD >= 1024
    assert KT % 2 == 0, "DoubleRow consumes K=128 subtiles in pairs"
    KP = KT // 2
    widths = _chunk_plan(NPAD)
    NCH = len(widths)
    CW = 8 * NCH  # candidates per row per core

    nc = bacc.Bacc(
        "TRN2", target_bir_lowering=False, debug=False, num_devices=NCORES
    )

    # chunk 1 can be split into two half-width DRAM params so its two DMAs
    # ride both HWDGE rings in parallel; with f[k45] moved off the sync ring
    # chunk 1 lands in time as one transfer, so the split stays off
    C1SPLIT = False
    f_t = nc.declare_dram_parameter("f_t", [P, KT * B], fp8, isOutput=False)
    bsegs = []
    for i, w in enumerate(widths):
        if i == 1 and C1SPLIT:
            bsegs.append(
                (
                    nc.declare_dram_parameter("bseg1a", [P, KT * 512], fp8, isOutput=False),
                    nc.declare_dram_parameter("bseg1b", [P, KT * 512], fp8, isOutput=False),
                )
            )
        else:
            bsegs.append(
                nc.declare_dram_parameter(f"bseg{i}", [P, KT * w], fp8, isOutput=False)
            )
    out = nc.declare_dram_parameter("cand", [P, MT * CW], f32, isOutput=True)

    # f is stored half-major in DRAM ([P, NH, KT, BH] flattened) so every
    # (B-half, k-pair) slice is one contiguous-per-partition DMA; the m-group
    # loop consumes exactly one half per group, so the halves stream in the
    # order the PE needs them with no oversized gating transfer
    BH = 4 * P if B % (4 * P) == 0 else B
    NH = B // BH
    MH = BH // P

    with tile.TileContext(nc) as tc, ExitStack() as ctx:
        cpool = ctx.enter_context(tc.tile_pool(name="const", bufs=1))
        ppool = ctx.enter_context(tc.tile_pool(name="psum", bufs=4, space="PSUM"))

        f_view = f_t.rearrange("p (h kt b) -> p h kt b", h=NH, kt=KT)
        bviews = [
            tuple(h.rearrange("p (kt n) -> p kt n", kt=KT) for h in s)
            if isinstance(s, tuple)
            else s.rearrange("p (kt n) -> p kt n", kt=KT)
            for s in bsegs
        ]

        # PE warm-up during the initial DMA wait: garbage matmuls on a
        # zeroed tile keep the clock ramping until real work arrives.
        warm = cpool.tile([P, 512], bf16, tag="warm")
        nc.gpsimd.memset(warm[:], 0.0)
        wpsum = ppool.tile([P, 1024], f32, tag="pt")  # borrow a pt slot
        for _ in range(N_WARMUP):
            nc.tensor.matmul(
                wpsum[:, :512], lhsT=warm[:, :P], rhs=warm[:], start=True, stop=True
            )

        # First wave, all contiguous-per-partition on the two HWDGE
        # queues:  sync: chunk0, f[k45]  /  scalar: f[k01], f[k23].
        # Later chunks alternate sync/scalar and queue FIFO behind.
        # per chunk: list of (tile, col_lo) pieces, each an exact-width SBUF
        # tile so every DMA destination is contiguous per partition
        btiles = []
        for i, w in enumerate(widths):
            if i == 1 and C1SPLIT:
                btiles.append(
                    [
                        (cpool.tile([P, KT, 512], fp8, tag="bt1a", name="bt1a"), 0),
                        (cpool.tile([P, KT, 512], fp8, tag="bt1b", name="bt1b"), 512),
                    ]
                )
            else:
                btiles.append(
                    [(cpool.tile([P, KT, w], fp8, tag=f"bt{i}", name=f"bt{i}"), 0)]
                )
        ftile = cpool.tile([P, NH, KT, BH], fp8, tag="ft", name="ft")

        # sync ring:   chunk0, f[k45,h0], f[k45,h1], chunk1, chunk3, ...
        # scalar ring: f[k01,h0], f[k23,h0], f[k01,h1], f[k23,h1], chunk2, ...
        # Each f transfer is one (B-half, k-pair) slice (~131KB, 1KB lines),
        # ordered exactly as the grouped chunk-0 loop consumes them: group 0
        # (m0-3) needs only the h0 slices, which all land by ~11.5us, so the
        # PE starts on chunk0+k01 with no further feature stalls.  Measured
        # orderings with k45 behind the other f slices on one ring stalled
        # the PE 1.6-2.7us.
        nc.sync.dma_start(btiles[0][0][0][:], bviews[0][:])
        for h in range(NH):
            for j in range(KP - 1):
                nc.scalar.dma_start(
                    ftile[:, h, 2 * j : 2 * j + 2, :],
                    f_view[:, h, 2 * j : 2 * j + 2, :],
                )
        for h in range(NH):
            jl = 2 * (KP - 1)
            nc.sync.dma_start(
                ftile[:, h, jl : jl + 2, :], f_view[:, h, jl : jl + 2, :]
            )
        for i in range(1, NCH):
            if i == 1 and C1SPLIT:
                nc.sync.dma_start(btiles[1][0][0][:], bviews[1][0][:])
                nc.scalar.dma_start(btiles[1][1][0][:], bviews[1][1][:])
            else:
                q = nc.sync if i % 2 == 1 else nc.scalar
                q.dma_start(btiles[i][0][0][:], bviews[i][:])

        # chunk-major output tile: chunk ci's 8*MT candidate columns are
        # complete right after its m-loop, so each chunk's slice DMAs out
        # mid-run and only the last chunk's slice gates the end
        otile = cpool.tile([P, NCH * MT * 8], f32, tag="cand", name="cand")

        def chalves(W):
            out_, lo = [], 0
            while lo < W:
                out_.append((lo, min(512, W - lo)))
                lo += 512
            return out_

        def mm(pt, ci, j, m, hlo, hw, start, stop):
            tile_, lo = next(
                (t, lo)
                for t, lo in reversed(btiles[ci])
                if lo <= hlo and hlo + hw <= lo + t.shape[2]
            )
            ml = m % MH
            nc.tensor.matmul(
                pt[:, hlo : hlo + hw],
                lhsT=ftile[:, m // MH, 2 * j : 2 * j + 2, ml * P : (ml + 1) * P],
                rhs=tile_[:, 2 * j : 2 * j + 2, hlo - lo : hlo - lo + hw],
                start=start,
                stop=stop,
                perf_mode=mybir.MatmulPerfMode.DoubleRow,
            )

        # chunk 0: j outer / m inner in m-groups of 4 (the PSUM bufs),
        # matching the k-pair arrival order so the PE starts as soon as
        # chunk0 + f[k01] land instead of waiting for the whole ftile.
        W0 = widths[0]
        GRP = min(MH, 4)  # psum pool has 4 bufs
        pts0 = {}
        for g in range(0, MT, GRP):
            ms = range(g, min(g + GRP, MT))
            for j in range(KP):
                for m in ms:
                    if j == 0:
                        pts0[m] = ppool.tile([P, 1024], f32, tag="pt", name=f"pt0_{m}")
                    for hlo, hw in chalves(W0):
                        mm(pts0[m], 0, j, m, hlo, hw, j == 0, j == KP - 1)
            for m in ms:
                nc.vector.max(otile[:, m * 8 : m * 8 + 8], pts0[m][:, :W0])
        nc.sync.dma_start(out[:, : MT * 8], otile[:, : MT * 8])

        # chunks 1+: plain per-m j-loop (data long since resident); each
        # chunk's candidate slice DMAs out mid-run so only the last (small)
        # slice gates the end
        for ci in range(1, NCH):
            W = widths[ci]
            for m in range(MT):
                pt = ppool.tile([P, 1024], f32, tag="pt", name=f"pt{ci}_{m}")
                for j in range(KP):
                    for hlo, hw in chalves(W):
                        mm(pt, ci, j, m, hlo, hw, j == 0, j == KP - 1)
                base = ci * MT * 8 + m * 8
                nc.vector.max(otile[:, base : base + 8], pt[:, :W])
            oq = nc.sync if ci % 2 == 0 else nc.scalar
            oq.dma_start(
                out[:, ci * MT * 8 : (ci + 1) * MT * 8],
                otile[:, ci * MT * 8 : (ci + 1) * MT * 8],
            )

    nc.finalize()
    return nc


def _seg(bT, KT, off, W):
    """Columns [off, off+W) of [D, cols] -> tile layout [P, KT*W]."""
    D = bT.shape[0]
    P_ = P
    s = bT[:, off : off + W].reshape(KT, P_, W)
    return np.ascontiguousarray(s.transpose(1, 0, 2).reshape(P_, KT * W))


def _tile_f(fT, KT):
    """[D, B] -> half-major tile layout [P, NH*KT*BH] (d = kt*P + p): B is
    split into 512-column halves so each (half, k-pair) DRAM slice is one
    contiguous-per-partition DMA."""
    B = fT.shape[1]
    BH = 4 * P if B % (4 * P) == 0 else B
    return np.concatenate(
        [_seg(fT, KT, h, BH) for h in range(0, B, BH)], axis=1
    )


def _unseg(arr, KT, W):
    """Tile layout [P, KT*W] -> [D, W] (inverse of _seg, for the sim check)."""
    return arr.reshape(P, KT, W).transpose(1, 0, 2).reshape(KT * P, W)


def _tile_b(bT, KT, widths):
    """[D, NPAD] -> dict of per-chunk tile-layout arrays bseg{i}."""
    out = {}
    off = 0
    for i, w in enumerate(widths):
        out[f"bseg{i}"] = _seg(bT, KT, off, w)
        off += w
    return out


def _shard_geom(N):
    NSH = -(-N // NCORES)
    NPAD = max(NSH, 1024)
    return NSH, NPAD, _chunk_plan(NPAD)


def _host_prep(features, memory_bank):
    """Shard + lay out inputs for the 8 cores."""
    import ml_dtypes

    f8 = ml_dtypes.float8_e4m3
    B, D = features.shape
    N = memory_bank.shape[0]
    NSH, NPAD, widths = _shard_geom(N)
    KT = D // P

    # Cross term uses data dims 0..D-2 only; row D-1 carries the m-norm:
    #   v = f[:D-1].m[:D-1] + 8 * fp8((C_M - |m|^2/2) / 8)
    fT = np.ascontiguousarray(features.T).astype(f8)
    fT[D - 1] = f8(8.0)
    x_sq = np.einsum("bd,bd->b", features, features, dtype=np.float32)
    f_tiled = _tile_f(fT, KT)

    msq = np.einsum("nd,nd->n", memory_bank, memory_bank, dtype=np.float32)

    in_maps = []
    for i in range(NCORES):
        lo = i * NSH
        hi = min(lo + NSH, N)
        n_i = hi - lo
        bT = np.zeros((D, NPAD), f8)
        bT[:, :n_i] = memory_bank[lo:hi].T.astype(f8)
        q_m = np.full(NPAD, -240.0, np.float32)  # pads: v = -1920, never top-8
        q_m[:n_i] = (C_M - 0.5 * msq[lo:hi]) / 8.0
        bT[D - 1] = q_m.astype(f8)
        im = {"f_t": f_tiled}
        im.update(_tile_b(bT, KT, widths))
        in_maps.append(im)
    return in_maps, NPAD, x_sq, msq


# test.py can flip these to get a profiled run
TRACE = False
LAST_RESULT = None
N_RECOMPUTED = 0


def _install_ntff_hook():
    """This container's `antenv` lacks `axon_hooks`; synthesize it so
    run_bass_kernel_spmd(trace=True) can profile via the axon .so."""
    import sys as _sys

    if "antenv.axon_hooks" in _sys.modules:
        return
    import contextlib, ctypes, types

    mod = types.ModuleType("antenv.axon_hooks")
    mod._hook = None
    mod.set_axon_ntff_profile_hook = lambda h: setattr(mod, "_hook", h)
    mod.get_axon_ntff_profile_hook = lambda: mod._hook

    so_path = "/opt/axon/libaxon_pjrt.so"
    try:
        lib = ctypes.CDLL(so_path)
        lib.axon_start_nrt_profile.argtypes = [
            ctypes.POINTER(ctypes.c_int64),
            ctypes.c_size_t,
        ]
        lib.axon_start_nrt_profile.restype = ctypes.c_int64
        lib.axon_stop_nrt_profile.argtypes = [ctypes.c_char_p]
        lib.axon_stop_nrt_profile.restype = ctypes.c_int64

        @contextlib.contextmanager
        def _hook(output_dir, device_ids):
            import jax

            jax.devices()
            if device_ids:
                ids = (ctypes.c_int64 * len(device_ids))(*device_ids)
                rc = lib.axon_start_nrt_profile(ids, len(device_ids))
            else:
                rc = lib.axon_start_nrt_profile(None, 0)
            if rc != 0:
                raise RuntimeError(f"axon_start_nrt_profile rc={rc}")
            try:
                yield
            finally:
                n = lib.axon_stop_nrt_profile(str(output_dir).encode())
                print(f"profile: {n} file(s) written to {output_dir}")

        mod._hook = _hook
    except (OSError, AttributeError):
        pass

    import antenv

    _sys.modules["antenv.axon_hooks"] = mod
    antenv.axon_hooks = mod


def _exact_row_scores(features, memory_bank, rows, kk):
    """Exact numpy top-k mean distance for a few suspect rows."""
    f = features[rows]  # [R, D]
    d2 = (
        np.einsum("rd,rd->r", f, f)[:, None]
        + np.einsum("nd,nd->n", memory_bank, memory_bank)[None, :]
        - 2.0 * (f @ memory_bank.T)
    )
    d2k = np.sort(d2, axis=1)[:, :kk]
    return np.sqrt(np.maximum(d2k, 0.0)).mean(axis=1)


def kernel(features, memory_bank, k):
    global LAST_RESULT, N_RECOMPUTED
    from concourse.bass_utils import run_bass_kernel_spmd

    features = np.asarray(features, dtype=np.float32)
    memory_bank = np.asarray(memory_bank, dtype=np.float32)
    B, D = features.shape
    N = memory_bank.shape[0]
    kk = min(int(k), N)
    if kk <= 0:
        # mean over an empty candidate set (matches jnp.mean of empty)
        return np.full(B, np.nan, np.float32)

    in_maps, NPAD, x_sq, msq = _host_prep(features, memory_bank)
    nc = _build(B, D, NPAD)

    if TRACE:
        _install_ntff_hook()
    res = run_bass_kernel_spmd(nc, in_maps, list(range(NCORES)), trace=TRACE)
    LAST_RESULT = res

    # gather per-(core, chunk) top-8 candidates; larger v = closer
    MT = B // P
    v = np.concatenate(
        [_untile_cand(res.results[i]["cand"], MT) for i in range(NCORES)], axis=1
    )  # [B, NCORES * 8 * nchunks]
    return _finalize(v, x_sq, features, memory_bank, kk)


def _untile_cand(arr, MT):
    """Device cand layout [P, NCH*MT*8] (chunk-major) -> [B, NCH*8]."""
    NCH = arr.shape[1] // (MT * 8)
    return arr.reshape(P, NCH, MT, 8).transpose(2, 0, 1, 3).reshape(MT * P, NCH * 8)


def _finalize(v, x_sq, features, memory_bank, kk):
    """Reduce the per-(core, chunk) top-8 candidates to the final scores."""
    global N_RECOMPUTED
    kk_c = min(kk, v.shape[1])
    order = np.argsort(-v, axis=1)[:, :kk_c]  # observed top-k candidates
    vk = np.take_along_axis(v, order, axis=1)
    # v = f.m_trunc + C_M - |m|^2/2  =>  d^2 = x_sq + 2*C_M - 2*v
    d = np.sqrt(np.maximum(x_sq[:, None] + 2.0 * C_M - 2.0 * vk, 0.0))
    scores = d.mean(axis=1).astype(np.float32)

    # A true top-k member can only be missing if >=8 elements of its
    # column chunk outrank it; then >=8 of the observed top-k come
    # from that chunk (index group of 8).  Recompute such rows exactly.
    N_RECOMPUTED = 0
    if kk >= 9:
        if kk > v.shape[1]:  # more than the candidate pool: all rows exact
            suspects = np.arange(v.shape[0])
        else:
            grp = np.sort(order // 8, axis=1)
            same8 = (grp[:, 7:] == grp[:, : grp.shape[1] - 7]).any(axis=1)
            suspects = np.nonzero(same8)[0]
        if suspects.size:
            N_RECOMPUTED = suspects.size
            scores[suspects] = _exact_row_scores(
                features, memory_bank, suspects, kk
            ).astype(np.float32)

    return scores
